# revision 24
# baseline (speedup 1.0000x reference)
"""GraphSAGE-max (3 layers + 2 heads) on 8 Trainium2 NeuronCores.

Strategy: data-parallel over dst-node partitions with replicated weights.
Nodes are dealt to the 8 cores snake-wise by in-degree, then re-sorted
inside each core by (in-degree, lo-half-degree) so a dense ELL gather
schedule has little padding. Features live in replicated DRAM tables of
bf16 rows; each core's table block carries its own -inf pad row so both
halves of the table are addressable with int16 dma_gather indices.

Key implementation points (v2):
  - Gather calls round-robin over 4 SWDGE queues (4 Q7 core pairs
    generate descriptors concurrently; measured ~3x on descriptor-bound
    gathers) and use deep tile pools so many calls stay in flight.
    (Node-major, non-transpose gathers: concurrent transpose-mode
    gathers on different queues corrupt each other via the shared XBAR.)
  - Per-dst max over ELL columns: one contiguous pair-fold on DVE, then
    a single strided tensor_reduce per gather span; a lo/hi-balanced
    node-to-core assignment shrinks the shared ELL padding.
  - All matmuls run in bf16 (f32 PSUM accumulate).
  - One shared gather schedule for the three aggregations: the int16
    index stream is loaded to SBUF once and reused by L1/L2/L3.
  - AllGather of bf16 blocks between layers; the two output heads share
    the third aggregation.
"""

import numpy as np
import ml_dtypes

import concourse.bass as bass
import concourse.bacc as bacc
import concourse.mybir as mybir
import concourse.tile as tile
from concourse.masks import make_identity
from concourse.bass_utils import run_bass_kernel_spmd

N = 50000
E = 800000
F_IN = 128
H = 256
NCOR = 8
NLOC = N // NCOR             # 6250
BLOCK = NLOC + 1             # 6251 rows per core block (last = -inf pad)
HALF = 4 * BLOCK             # 25004 rows per table half
TILES = (NLOC + 127) // 128  # 49
PADN = TILES * 128           # 6272
NEG = float(np.finfo(np.float32).min)
KCOL = 16                    # max gather columns (of 128 idx) per call
CHUNK = 4                    # node tiles per matmul chunk (N free = 512)
NQUEUES = 4                  # SWDGE queues used round-robin
PADIDX = NLOC                # pad row local index inside a table half

_LAST = {}                   # stash for the test harness


# ----------------------------------------------------------------------------
# host-side graph preprocessing
# ----------------------------------------------------------------------------

def _preprocess(edge_index):
    src = np.asarray(edge_index[0], np.int64)
    dst = np.asarray(edge_index[1], np.int64)
    deg = np.bincount(dst, minlength=N)

    # deal nodes (by degree desc) to cores snake-wise -> owner per old id
    order = np.argsort(-deg, kind="stable")
    ranks = np.arange(N)
    pos = ranks % NCOR
    core_of_rank = np.where((ranks // NCOR) % 2 == 0, pos, NCOR - 1 - pos)
    owner = np.empty(N, np.int64)
    owner[order] = core_of_rank

    # Balance each dst's lo/hi neighbor split with equal-degree owner
    # swaps between lo cores and hi cores (preserves per-core degree
    # profiles; shrinks the shared ELL k-padding by ~10%).
    for _ in range(12):
        lo_of = owner < 4
        dlo_b = np.bincount(dst[lo_of[src]], minlength=N)
        dhi_b = deg - dlo_b
        cur = np.maximum(dlo_b, dhi_b)
        newlo = np.where(lo_of[src], dlo_b[dst] - 1, dlo_b[dst] + 1)
        newhi = deg[dst] - newlo
        delta_e = np.maximum(newlo, newhi) - cur[dst]
        gain = -np.bincount(src, weights=delta_e, minlength=N)
        moved = 0
        for dv in range(0, int(deg.max()) + 1):
            cand = np.where(deg == dv)[0]
            if not len(cand):
                continue
            lo_c = cand[(owner[cand] < 4) & (gain[cand] > 0)]
            hi_c = cand[(owner[cand] >= 4) & (gain[cand] > 0)]
            k = min(len(lo_c), len(hi_c))
            if not k:
                continue
            lo_c = lo_c[np.argsort(-gain[lo_c])][:k]
            hi_c = hi_c[np.argsort(-gain[hi_c])][:k]
            k = max(1, int(k * 0.5))
            lo_c, hi_c = lo_c[:k], hi_c[:k]
            tmp = owner[lo_c].copy()
            owner[lo_c] = owner[hi_c]
            owner[hi_c] = tmp
            moved += k
        if moved < 50:
            break

    # lo half = nodes owned by cores 0-3
    lo_of_old = owner < 4
    deg_lo = np.bincount(dst[lo_of_old[src]], minlength=N)

    # within-core order: (deg desc, lo-deg desc) -> tight ELL
    old_of_new = np.empty(N, np.int64)
    for m in range(NCOR):
        nodes = np.where(owner == m)[0]
        key = np.lexsort((-deg_lo[nodes], -deg[nodes]))
        old_of_new[m * NLOC:(m + 1) * NLOC] = nodes[key]
    new_of_old = np.empty(N, np.int64)
    new_of_old[old_of_new] = np.arange(N)

    # local index within the table half, per old id
    m_of_old = new_of_old // NLOC
    r_of_old = new_of_old % NLOC
    tloc_of_old = np.where(m_of_old < 4, m_of_old, m_of_old - 4) * BLOCK + r_of_old

    # per-dst phase-split neighbor slots
    nd = new_of_old[dst]
    ph = (~lo_of_old[src]).astype(np.int64)           # 0 = lo, 1 = hi
    stloc = tloc_of_old[src]
    gk = nd * 2 + ph
    eorder = np.lexsort((stloc, gk))
    gk_s = gk[eorder]
    stloc_s = stloc[eorder]
    starts = np.searchsorted(gk_s, np.arange(2 * N))
    slot = np.arange(E) - starts[gk_s]
    cnt = np.bincount(gk, minlength=2 * N)
    dlo = cnt[0::2]                  # lo-degree, indexed by new id
    dhi = cnt[1::2]

    # shared compile-time K schedule per (tile, phase): max over cores
    def ktile(d):
        ks = np.zeros(TILES, np.int64)
        for m in range(NCOR):
            dm = d[m * NLOC:(m + 1) * NLOC]
            for t in range(TILES):
                blk = dm[t * 128:(t + 1) * 128]
                if blk.size:
                    ks[t] = max(ks[t], int(blk.max()))
        return np.maximum(ks, 1)
    klo = ktile(dlo)
    khi = ktile(dhi)

    # dense ELL per phase [NCOR, PADN, kmax]
    def ell_of(phase, kmax):
        ell = np.full((NCOR, PADN, kmax), PADIDX, np.int16)
        sel = ph[eorder] == phase
        nd_e = nd[eorder][sel]
        ell[nd_e // NLOC, nd_e % NLOC, slot[sel]] = stloc_s[sel].astype(np.int16)
        return ell
    ells = {0: ell_of(0, int(klo.max())), 1: ell_of(1, int(khi.max()))}

    # -------- call schedule --------------------------------------------
    # Calls live inside one (chunk, phase). Each call covers <= KCOL
    # columns; a column is 128 indices (k-major inside a tile span):
    # span (t, k0, kn) contributes kn columns [idx(k,slot)].
    NCH = (TILES + CHUNK - 1) // CHUNK
    calls = []        # (phase, col0_global, ncols, chunk, [(t, k0, kn, spanoff)])
    idx_blocks = []   # per call: wrapped int16 [NCOR, 128, ncols]
    col_off = 0
    for c in range(NCH):
        tlist = range(c * CHUNK, min((c + 1) * CHUNK, TILES))
        for phase, ks in ((0, klo), (1, khi)):
            pend_spans = []
            pend_cols = 0

            def flush():
                nonlocal pend_spans, pend_cols, col_off
                if not pend_cols:
                    return
                blk = np.empty((NCOR, 128, pend_cols), np.int16)
                for (t, k0, kn, off) in pend_spans:
                    e = ells[phase][:, t * 128:(t + 1) * 128, k0:k0 + kn]
                    # [NCOR, 128slot, kn] -> columns k-major
                    blk[:, :, off:off + kn] = e
                calls.append((phase, col_off, pend_cols, c,
                              list(pend_spans)))
                idx_blocks.append(blk)
                col_off += pend_cols
                pend_spans = []
                pend_cols = 0

            for t in tlist:
                k = int(ks[t])
                k0 = 0
                while k0 < k:
                    kn = min(KCOL - pend_cols, k - k0)
                    pend_spans.append((t, k0, kn, pend_cols))
                    pend_cols += kn
                    k0 += kn
                    if pend_cols == KCOL:
                        flush()
            flush()
    total_cols = col_off

    # wrapped idx stream: per call, column-major-by-16-partition wrap.
    # A call's indices flat order: for col (k within span), slot s:
    # flat = col*128 + s; wrapped [16, cols*8] then replicated to 128.
    idx_flat = np.empty((NCOR, 128, total_cols * 8), np.int16)
    for call_i, (phase, col0, ncols, c, spans) in enumerate(calls):
        blk = idx_blocks[call_i]                    # [NCOR, 128slot, ncols]
        flat = blk.transpose(0, 2, 1).reshape(NCOR, ncols * 128)
        nidx = ncols * 128
        w = np.zeros((NCOR, 16, nidx // 16), np.int16)
        i = np.arange(nidx)
        w[:, i % 16, i // 16] = flat
        idx_flat[:, :, col0 * 8:(col0 + ncols) * 8] = np.tile(w, (1, 8, 1))
    idx_flat = idx_flat.reshape(NCOR, 128 * total_cols * 8)

    return dict(new_of_old=new_of_old, old_of_new=old_of_new,
                calls=calls, total_cols=total_cols, idx_flat=idx_flat)


# ----------------------------------------------------------------------------
# device program
# ----------------------------------------------------------------------------

def _build_program(calls, total_cols):
    nc = bacc.Bacc("TRN2", target_bir_lowering=False, debug=False,
                   num_devices=NCOR, num_swdge_queues=NQUEUES)
    f32, bf16, i16 = mybir.dt.float32, mybir.dt.bfloat16, mybir.dt.int16

    t_xtab = nc.dram_tensor("xtab", [2 * HALF, F_IN], bf16,
                            kind="ExternalInput")
    t_xT = nc.dram_tensor("xT", [128, PADN], bf16, kind="ExternalInput")
    t_idx = nc.dram_tensor("idx", [128 * total_cols * 8], i16,
                           kind="ExternalInput")
    wnames = ["Wl1", "Wr1", "Wl2", "Wr2", "Wla", "Wra", "Wlm", "Wrm"]
    wcols = {"Wl1": H, "Wr1": H}
    t_w = {w: nc.dram_tensor(w, [128, wcols.get(w, 2 * H)], bf16,
                             kind="ExternalInput") for w in wnames}
    t_b = {b: nc.dram_tensor(b, [128, 2], f32, kind="ExternalInput")
           for b in ["bl1", "bl2", "bla", "blm"]}
    t_wh = {w: nc.dram_tensor(w, [128, 2], bf16, kind="ExternalInput")
            for w in ["Wa", "Wm"]}
    t_bh = {b: nc.dram_tensor(b, [1, 1], f32, kind="ExternalInput")
            for b in ["ba", "bm"]}
    t_out = nc.dram_tensor("out", [2, NLOC], f32, kind="ExternalOutput")

    NCH = (TILES + CHUNK - 1) // CHUNK
    cw_of = lambda c: min(CHUNK, TILES - c * CHUNK) * 128
    calls_of_chunk = {}
    for ci, (phase, col0, ncols, c, spans) in enumerate(calls):
        calls_of_chunk.setdefault(c, []).append(ci)

    qctr = [0]

    with tile.TileContext(nc) as tc:
        with tc.tile_pool(name="const", bufs=1) as cpool, \
             tc.tile_pool(name="hT", bufs=1) as hpool, \
             tc.tile_pool(name="gat", bufs=8) as gp, \
             tc.tile_pool(name="work", bufs=3) as wk, \
             tc.tile_pool(name="psT", bufs=3, space="PSUM") as psT, \
             tc.tile_pool(name="psY", bufs=3, space="PSUM") as psY, \
             tc.tile_pool(name="dram", bufs=1, space="DRAM") as dram:

            ident = cpool.tile([128, 128], bf16, name="ident")
            make_identity(nc, ident[:])

            w_sb = {}
            for w in wnames:
                cols = wcols.get(w, 2 * H)
                ws = cpool.tile([128, cols], bf16, name=f"sb_{w}")
                nc.sync.dma_start(ws[:], t_w[w][:])
                w_sb[w] = ws
            b_sb = {}
            for b in t_b:
                bs = cpool.tile([128, 2], f32, name=f"sb_{b}")
                nc.sync.dma_start(bs[:], t_b[b][:])
                b_sb[b] = bs
            wh_sb = {}
            for w in t_wh:
                ws = cpool.tile([128, 2], bf16, name=f"sb_{w}")
                nc.sync.dma_start(ws[:], t_wh[w][:])
                wh_sb[w] = ws
            bh_sb = {}
            for b in t_bh:
                bs = cpool.tile([1, 1], f32, name=f"sb_{b}")
                nc.sync.dma_start(bs[:], t_bh[b][:])
                bh_sb[b] = bs

            # shared gather index stream (same for all three layers).
            # Loaded before weights/xT: it gates the first gather. The
            # first chunk's columns come in a small leading DMA.
            idx_sb = hpool.tile([128, total_cols * 8], i16, name="idx_sb")
            c0_cols = max(c[1] + c[2] for c in calls if c[3] == 0) * 8
            idx2d = t_idx[:].rearrange("(p s) -> p s", p=128)
            nc.sync.dma_start(idx_sb[:, :c0_cols], idx2d[:, :c0_cols])
            nc.sync.dma_start(idx_sb[:, c0_cols:], idx2d[:, c0_cols:])

            xT_sb = hpool.tile([128, PADN], bf16, name="xT_sb")
            nc.sync.dma_start(xT_sb[:], t_xT[:])
            h1T = hpool.tile([128, 2 * PADN], bf16, name="h1T")
            h2T = hpool.tile([128, 2 * PADN], bf16, name="h2T")

            h1tab = dram.tile([2 * HALF, H], bf16, name="h1tab",
                              addr_space="Shared")
            h2tab = dram.tile([2 * HALF, H], bf16, name="h2tab",
                              addr_space="Shared")
            blk1 = dram.tile([BLOCK, H], bf16, name="blk1")
            blk2 = dram.tile([BLOCK, H], bf16, name="blk2")

            # each core's block ends with a -inf pad row
            padrow = cpool.tile([1, H], bf16, name="padrow")
            nc.vector.memset(padrow[:], NEG)
            nc.sync.dma_start(blk1[NLOC:NLOC + 1, :], padrow[:])
            nc.sync.dma_start(blk2[NLOC:NLOC + 1, :], padrow[:])

            def aggregate_chunk(c, table, F, tag):
                """Gather + max-reduce chunk c's neighbors from `table`.
                Node-major gathers (multi-queue safe), per-span strided
                DVE max-reduce, then PE-transpose the reduced [128, F]
                aggregate. Returns bf16 f-major agg tile [128, fh*512]."""
                fh = F // 128
                agg = wk.tile([128, fh * 512], bf16, name=f"agg_{tag}",
                              tag="agg")
                aggn = {}        # tile -> node-major agg tile [128, F]
                for ci in calls_of_chunk[c]:
                    phase, col0, ncols, _, spans = calls[ci]
                    nidx = ncols * 128
                    g = gp.tile([128, KCOL * F], bf16,
                                name=f"g_{tag}_{ci}", tag="g")
                    view = table[0:HALF, :] if phase == 0 \
                        else table[HALF:2 * HALF, :]
                    nc.gpsimd.dma_gather(
                        out_ap=g[:, :ncols * F].rearrange(
                            "p (k f) -> p k f", f=F),
                        in_ap=view,
                        idxs_ap=idx_sb[:, col0 * 8:(col0 + ncols) * 8],
                        num_idxs=nidx, num_idxs_reg=nidx, elem_size=F,
                        single_packet=False,
                        queue_num=qctr[0] % NQUEUES)
                    qctr[0] += 1
                    for (t, k0, kn, off) in spans:
                        # one contiguous pair-fold (halves the columns),
                        # then a single strided max-reduce over the rest
                        k = kn
                        if k > 2 and k % 2 == 1:
                            nc.vector.tensor_tensor(
                                out=g[:, off * F:(off + 1) * F],
                                in0=g[:, off * F:(off + 1) * F],
                                in1=g[:, (off + k - 1) * F:(off + k) * F],
                                op=mybir.AluOpType.max)
                            k -= 1
                        if k > 2:
                            half = k // 2
                            nc.vector.tensor_tensor(
                                out=g[:, off * F:(off + half) * F],
                                in0=g[:, off * F:(off + half) * F],
                                in1=g[:, (off + half) * F:
                                       (off + 2 * half) * F],
                                op=mybir.AluOpType.max)
                            k = half
                        src = g[:, off * F:(off + k) * F]
                        view3 = src.rearrange("p (k f) -> p f k", f=F)
                        if t not in aggn:
                            an = wk.tile([128, F], bf16,
                                         name=f"an_{tag}_{t}", tag="aggn",
                                         bufs=10)
                            aggn[t] = an
                            nc.vector.tensor_reduce(
                                out=an[:], in_=view3,
                                axis=mybir.AxisListType.X,
                                op=mybir.AluOpType.max)
                        else:
                            an = aggn[t]
                            tmp = wk.tile([128, F], bf16,
                                          name=f"tmp_{tag}", tag="tmp")
                            nc.vector.tensor_reduce(
                                out=tmp[:], in_=view3,
                                axis=mybir.AxisListType.X,
                                op=mybir.AluOpType.max)
                            nc.vector.tensor_tensor(
                                out=an[:], in0=an[:], in1=tmp[:],
                                op=mybir.AluOpType.max)
                for t, an in aggn.items():
                    toff = (t - c * CHUNK) * 128
                    for p in range(fh):
                        tp = psT.tile([128, 128], bf16, name=f"tpa_{tag}",
                                      tag="tp")
                        nc.tensor.transpose(
                            tp[:], an[:, p * 128:(p + 1) * 128], ident[:])
                        nc.scalar.activation(
                            agg[:, p * 512 + toff:p * 512 + toff + 128],
                            tp[:],
                            mybir.ActivationFunctionType.Identity)
                return agg

            def write_table(yT, c, blkout, tag):
                """yT bf16 [128, 2*PADN] planes -> node-major rows of blkout
                for chunk c (PE transpose + ACT copy)."""
                cw = cw_of(c)
                for i in range(cw // 128):
                    t = c * CHUNK + i
                    ynode = wk.tile([128, H], bf16, name=f"yn_{tag}",
                                    tag="ynode")
                    for hh in range(2):
                        tp = psT.tile([128, 128], bf16, name=f"tpo_{tag}",
                                      tag="tp")
                        nc.tensor.transpose(
                            tp[:],
                            yT[:, hh * PADN + t * 128:hh * PADN + (t + 1) * 128],
                            ident[:])
                        nc.scalar.activation(
                            ynode[:, hh * 128:(hh + 1) * 128], tp[:],
                            mybir.ActivationFunctionType.Identity)
                    rows = min(128, NLOC - t * 128)
                    nc.sync.dma_start(blkout[t * 128:t * 128 + rows, :],
                                      ynode[:rows, :])

            def layer(table, selfT, F, Wl, Wr, bl, outT, blkout, tag):
                fh = F // 128
                for c in range(NCH):
                    cw = cw_of(c)
                    c0 = c * CHUNK * 128
                    agg = aggregate_chunk(c, table, F, f"{tag}_{c}")
                    for hh in range(2):
                        psy = psY.tile([128, 512], f32, name=f"psy_{tag}",
                                       tag="psy")
                        nmm = 2 * fh
                        i = 0
                        for p in range(fh):
                            nc.tensor.matmul(
                                psy[:, :cw],
                                w_sb[Wl][:, p * H + hh * 128:
                                         p * H + (hh + 1) * 128],
                                agg[:, p * 512:p * 512 + cw],
                                start=(i == 0), stop=(i == nmm - 1))
                            i += 1
                            nc.tensor.matmul(
                                psy[:, :cw],
                                w_sb[Wr][:, p * H + hh * 128:
                                         p * H + (hh + 1) * 128],
                                selfT[:, p * PADN + c0:p * PADN + c0 + cw],
                                start=(i == 0), stop=(i == nmm - 1))
                            i += 1
                        nc.scalar.activation(
                            outT[:, hh * PADN + c0:hh * PADN + c0 + cw],
                            psy[:, :cw],
                            mybir.ActivationFunctionType.Relu,
                            bias=b_sb[bl][:, hh:hh + 1])
                    write_table(outT, c, blkout, tag)

            layer(t_xtab, xT_sb, F_IN, "Wl1", "Wr1", "bl1", h1T, blk1, "l1")
            nc.gpsimd.collective_compute(
                "AllGather", mybir.AluOpType.bypass,
                replica_groups=[list(range(NCOR))],
                ins=[blk1.opt()], outs=[h1tab.opt()])
            layer(h1tab, h1T, H, "Wl2", "Wr2", "bl2", h2T, blk2, "l2")
            nc.gpsimd.collective_compute(
                "AllGather", mybir.AluOpType.bypass,
                replica_groups=[list(range(NCOR))],
                ins=[blk2.opt()], outs=[h2tab.opt()])

            # layer 3: two branches + heads (shared aggregation)
            for c in range(NCH):
                cw = cw_of(c)
                c0 = c * CHUNK * 128
                agg = aggregate_chunk(c, h2tab, H, f"l3_{c}")
                out_sbs = [wk.tile([1, 512], f32, name=f"out_sb{bi}",
                                   tag=f"out_sb{bi}") for bi in range(2)]
                for bi, (Wl, Wr, bl, Wh, bh) in enumerate(
                        [("Wla", "Wra", "bla", "Wa", "ba"),
                         ("Wlm", "Wrm", "blm", "Wm", "bm")]):
                    brT = wk.tile([128, 2 * 512], bf16, name=f"brT{bi}",
                                  tag="brT")
                    for hh in range(2):
                        psy = psY.tile([128, 512], f32, name=f"psy3_{bi}",
                                       tag="psy")
                        for p in range(2):
                            nc.tensor.matmul(
                                psy[:, :cw],
                                w_sb[Wl][:, p * H + hh * 128:
                                         p * H + (hh + 1) * 128],
                                agg[:, p * 512:p * 512 + cw],
                                start=(p == 0), stop=False)
                            nc.tensor.matmul(
                                psy[:, :cw],
                                w_sb[Wr][:, p * H + hh * 128:
                                         p * H + (hh + 1) * 128],
                                h2T[:, p * PADN + c0:p * PADN + c0 + cw],
                                start=False, stop=(p == 1))
                        nc.scalar.activation(
                            brT[:, hh * 512:hh * 512 + cw], psy[:, :cw],
                            mybir.ActivationFunctionType.Relu,
                            bias=b_sb[bl][:, hh:hh + 1])
                    psh = psY.tile([1, 512], f32, name=f"psh{bi}", tag="psh",
                                   bufs=2)
                    for hh in range(2):
                        nc.tensor.matmul(psh[:, :cw],
                                         wh_sb[Wh][:, hh:hh + 1],
                                         brT[:, hh * 512:hh * 512 + cw],
                                         start=(hh == 0), stop=(hh == 1))
                    nc.scalar.activation(out_sbs[bi][:, :cw],
                                         psh[:, :cw],
                                         mybir.ActivationFunctionType.Identity,
                                         bias=bh_sb[bh][:])
                live = min(cw, NLOC - c0)
                for bi in range(2):
                    nc.sync.dma_start(
                        t_out[bi:bi + 1, c0:c0 + live],
                        out_sbs[bi][:, :live])

    nc.compile()
    return nc


# ----------------------------------------------------------------------------
# entry point
# ----------------------------------------------------------------------------

def kernel(x, edge_index, Wl1, bl1, Wr1, Wl2, bl2, Wr2,
           Wla, bla, Wra, Wa, ba, Wlm, blm, Wrm, Wm, bm):
    x = np.asarray(x, np.float32)
    pp = _preprocess(edge_index)
    old_of_new = pp["old_of_new"]

    # x gather table in block layout: per core 6250 rows + one -inf pad row
    xp = x[old_of_new]
    xtab = np.empty((2 * HALF, F_IN), np.float32)
    for m in range(NCOR):
        base = m * BLOCK if m < 4 else HALF + (m - 4) * BLOCK
        xtab[base:base + NLOC] = xp[m * NLOC:(m + 1) * NLOC]
        xtab[base + NLOC] = NEG
    xtab = xtab.astype(ml_dtypes.bfloat16)

    nc = _build_program(pp["calls"], pp["total_cols"])

    def wchunk(W, fi):
        """[fi, H] f32 -> bf16 [128, (fi//128)*H] chunk-major stationary."""
        W = np.asarray(W, np.float32)
        fh = fi // 128
        out = np.empty((128, fh * H), np.float32)
        for c in range(fh):
            out[:, c * H:(c + 1) * H] = W[c * 128:(c + 1) * 128, :]
        return out.astype(ml_dtypes.bfloat16)

    def bchunk(b):
        return np.ascontiguousarray(
            np.asarray(b, np.float32).reshape(2, 128).T)

    w_ins = {
        "Wl1": wchunk(Wl1, F_IN), "Wr1": wchunk(Wr1, F_IN),
        "Wl2": wchunk(Wl2, H), "Wr2": wchunk(Wr2, H),
        "Wla": wchunk(Wla, H), "Wra": wchunk(Wra, H),
        "Wlm": wchunk(Wlm, H), "Wrm": wchunk(Wrm, H),
        "Wa": np.asarray(Wa, np.float32).reshape(2, 128).T.astype(
            ml_dtypes.bfloat16).copy(),
        "Wm": np.asarray(Wm, np.float32).reshape(2, 128).T.astype(
            ml_dtypes.bfloat16).copy(),
        "bl1": bchunk(bl1), "bl2": bchunk(bl2),
        "bla": bchunk(bla), "blm": bchunk(blm),
        "ba": np.asarray(ba, np.float32).reshape(1, 1),
        "bm": np.asarray(bm, np.float32).reshape(1, 1),
    }

    in_maps = []
    for m in range(NCOR):
        blk = xp[m * NLOC:(m + 1) * NLOC]
        xT = np.zeros((128, PADN), np.float32)
        xT[:, :NLOC] = blk.T
        in_maps.append({
            "xtab": xtab, "xT": xT.astype(ml_dtypes.bfloat16),
            "idx": pp["idx_flat"][m], **w_ins,
        })

    res = run_bass_kernel_spmd(nc, in_maps, core_ids=list(range(NCOR)))

    rt = np.empty(N, np.float32)
    mv = np.empty(N, np.float32)
    for m in range(NCOR):
        out = res.results[m]["out"]
        rt[m * NLOC:(m + 1) * NLOC] = out[0]
        mv[m * NLOC:(m + 1) * NLOC] = out[1]
    rt_o = np.empty(N, np.float32)
    mv_o = np.empty(N, np.float32)
    rt_o[old_of_new] = rt
    mv_o[old_of_new] = mv

    _LAST.update(nc=nc, in_maps=in_maps, pp=pp)
    return (rt_o, mv_o)


# revision 25
# speedup vs baseline: 1.1106x; 1.1106x over previous
"""GraphSAGE-max (3 layers + 2 heads) on 8 Trainium2 NeuronCores.

Strategy: data-parallel over dst-node partitions with replicated weights.
Nodes are dealt to the 8 cores snake-wise by in-degree, then re-sorted
inside each core by (in-degree, lo-half-degree) so a dense ELL gather
schedule has little padding. Features live in replicated DRAM tables of
bf16 rows; each core's table block carries its own -inf pad row so both
halves of the table are addressable with int16 dma_gather indices.

Key implementation points (v2):
  - Gather calls round-robin over 4 SWDGE queues (4 Q7 core pairs
    generate descriptors concurrently; measured ~3x on descriptor-bound
    gathers) and use deep tile pools so many calls stay in flight.
    (Node-major, non-transpose gathers: concurrent transpose-mode
    gathers on different queues corrupt each other via the shared XBAR.)
  - Per-dst max over ELL columns: one contiguous pair-fold on DVE, then
    a single strided tensor_reduce per gather span; a lo/hi-balanced
    node-to-core assignment shrinks the shared ELL padding.
  - All matmuls run in bf16 (f32 PSUM accumulate).
  - One shared gather schedule for the three aggregations: the int16
    index stream is loaded to SBUF once and reused by L1/L2/L3.
  - AllGather of bf16 blocks between layers; the two output heads share
    the third aggregation.
"""

import numpy as np
import ml_dtypes

import concourse.bass as bass
import concourse.bacc as bacc
import concourse.mybir as mybir
import concourse.tile as tile
from concourse.masks import make_identity
from concourse.bass_utils import run_bass_kernel_spmd

N = 50000
E = 800000
F_IN = 128
H = 256
NCOR = 8
NLOC = N // NCOR             # 6250
BLOCK = NLOC + 1             # 6251 rows per core block (last = -inf pad)
HALF = 4 * BLOCK             # 25004 rows per table half
TILES = (NLOC + 127) // 128  # 49
PADN = TILES * 128           # 6272
NEG = float(np.finfo(np.float32).min)
KCOL = 16                    # max gather columns (of 128 idx) per call
CHUNK = 4                    # node tiles per matmul chunk (N free = 512)
NQUEUES = 4                  # SWDGE queues used round-robin
PADIDX = NLOC                # pad row local index inside a table half

_LAST = {}                   # stash for the test harness


# ----------------------------------------------------------------------------
# host-side graph preprocessing
# ----------------------------------------------------------------------------

def _preprocess(edge_index):
    src = np.asarray(edge_index[0], np.int64)
    dst = np.asarray(edge_index[1], np.int64)
    deg = np.bincount(dst, minlength=N)

    # deal nodes (by degree desc) to cores snake-wise -> owner per old id
    order = np.argsort(-deg, kind="stable")
    ranks = np.arange(N)
    pos = ranks % NCOR
    core_of_rank = np.where((ranks // NCOR) % 2 == 0, pos, NCOR - 1 - pos)
    owner = np.empty(N, np.int64)
    owner[order] = core_of_rank

    # Balance each dst's lo/hi neighbor split with equal-degree owner
    # swaps between lo cores and hi cores (preserves per-core degree
    # profiles; shrinks the shared ELL k-padding by ~10%).
    for _ in range(12):
        lo_of = owner < 4
        dlo_b = np.bincount(dst[lo_of[src]], minlength=N)
        dhi_b = deg - dlo_b
        cur = np.maximum(dlo_b, dhi_b)
        newlo = np.where(lo_of[src], dlo_b[dst] - 1, dlo_b[dst] + 1)
        newhi = deg[dst] - newlo
        delta_e = np.maximum(newlo, newhi) - cur[dst]
        gain = -np.bincount(src, weights=delta_e, minlength=N)
        moved = 0
        for dv in range(0, int(deg.max()) + 1):
            cand = np.where(deg == dv)[0]
            if not len(cand):
                continue
            lo_c = cand[(owner[cand] < 4) & (gain[cand] > 0)]
            hi_c = cand[(owner[cand] >= 4) & (gain[cand] > 0)]
            k = min(len(lo_c), len(hi_c))
            if not k:
                continue
            lo_c = lo_c[np.argsort(-gain[lo_c])][:k]
            hi_c = hi_c[np.argsort(-gain[hi_c])][:k]
            k = max(1, int(k * 0.5))
            lo_c, hi_c = lo_c[:k], hi_c[:k]
            tmp = owner[lo_c].copy()
            owner[lo_c] = owner[hi_c]
            owner[hi_c] = tmp
            moved += k
        if moved < 50:
            break

    # lo half = nodes owned by cores 0-3
    lo_of_old = owner < 4
    deg_lo = np.bincount(dst[lo_of_old[src]], minlength=N)

    # within-core order: (deg desc, lo-deg desc) -> tight ELL
    old_of_new = np.empty(N, np.int64)
    for m in range(NCOR):
        nodes = np.where(owner == m)[0]
        key = np.lexsort((-deg_lo[nodes], -deg[nodes]))
        old_of_new[m * NLOC:(m + 1) * NLOC] = nodes[key]
    new_of_old = np.empty(N, np.int64)
    new_of_old[old_of_new] = np.arange(N)

    # local index within the table half, per old id
    m_of_old = new_of_old // NLOC
    r_of_old = new_of_old % NLOC
    tloc_of_old = np.where(m_of_old < 4, m_of_old, m_of_old - 4) * BLOCK + r_of_old

    # per-dst phase-split neighbor slots
    nd = new_of_old[dst]
    ph = (~lo_of_old[src]).astype(np.int64)           # 0 = lo, 1 = hi
    stloc = tloc_of_old[src]
    gk = nd * 2 + ph
    eorder = np.lexsort((stloc, gk))
    gk_s = gk[eorder]
    stloc_s = stloc[eorder]
    starts = np.searchsorted(gk_s, np.arange(2 * N))
    slot = np.arange(E) - starts[gk_s]
    cnt = np.bincount(gk, minlength=2 * N)
    dlo = cnt[0::2]                  # lo-degree, indexed by new id
    dhi = cnt[1::2]

    # shared compile-time K schedule per (tile, phase): max over cores
    def ktile(d):
        ks = np.zeros(TILES, np.int64)
        for m in range(NCOR):
            dm = d[m * NLOC:(m + 1) * NLOC]
            for t in range(TILES):
                blk = dm[t * 128:(t + 1) * 128]
                if blk.size:
                    ks[t] = max(ks[t], int(blk.max()))
        return np.maximum(ks, 1)
    klo = ktile(dlo)
    khi = ktile(dhi)

    # dense ELL per phase [NCOR, PADN, kmax]
    def ell_of(phase, kmax):
        ell = np.full((NCOR, PADN, kmax), PADIDX, np.int16)
        sel = ph[eorder] == phase
        nd_e = nd[eorder][sel]
        ell[nd_e // NLOC, nd_e % NLOC, slot[sel]] = stloc_s[sel].astype(np.int16)
        return ell
    ells = {0: ell_of(0, int(klo.max())), 1: ell_of(1, int(khi.max()))}

    # -------- call schedule --------------------------------------------
    # Calls live inside one (chunk, phase). Each call covers <= KCOL
    # columns; a column is 128 indices (k-major inside a tile span):
    # span (t, k0, kn) contributes kn columns [idx(k,slot)].
    NCH = (TILES + CHUNK - 1) // CHUNK
    calls = []        # (phase, col0_global, ncols, chunk, [(t, k0, kn, spanoff)])
    idx_blocks = []   # per call: wrapped int16 [NCOR, 128, ncols]
    col_off = 0
    for c in range(NCH):
        tlist = range(c * CHUNK, min((c + 1) * CHUNK, TILES))
        for phase, ks in ((0, klo), (1, khi)):
            pend_spans = []
            pend_cols = 0

            def flush():
                nonlocal pend_spans, pend_cols, col_off
                if not pend_cols:
                    return
                blk = np.empty((NCOR, 128, pend_cols), np.int16)
                for (t, k0, kn, off) in pend_spans:
                    e = ells[phase][:, t * 128:(t + 1) * 128, k0:k0 + kn]
                    # [NCOR, 128slot, kn] -> columns k-major
                    blk[:, :, off:off + kn] = e
                calls.append((phase, col_off, pend_cols, c,
                              list(pend_spans)))
                idx_blocks.append(blk)
                col_off += pend_cols
                pend_spans = []
                pend_cols = 0

            for t in tlist:
                k = int(ks[t])
                k0 = 0
                while k0 < k:
                    kn = min(KCOL - pend_cols, k - k0)
                    pend_spans.append((t, k0, kn, pend_cols))
                    pend_cols += kn
                    k0 += kn
                    if pend_cols == KCOL:
                        flush()
            flush()
    total_cols = col_off

    # wrapped idx stream: per call, column-major-by-16-partition wrap.
    # A call's indices flat order: for col (k within span), slot s:
    # flat = col*128 + s; wrapped [16, cols*8] then replicated to 128.
    idx_flat = np.empty((NCOR, 128, total_cols * 8), np.int16)
    for call_i, (phase, col0, ncols, c, spans) in enumerate(calls):
        blk = idx_blocks[call_i]                    # [NCOR, 128slot, ncols]
        flat = blk.transpose(0, 2, 1).reshape(NCOR, ncols * 128)
        nidx = ncols * 128
        w = np.zeros((NCOR, 16, nidx // 16), np.int16)
        i = np.arange(nidx)
        w[:, i % 16, i // 16] = flat
        idx_flat[:, :, col0 * 8:(col0 + ncols) * 8] = np.tile(w, (1, 8, 1))
    idx_flat = idx_flat.reshape(NCOR, 128 * total_cols * 8)

    return dict(new_of_old=new_of_old, old_of_new=old_of_new,
                calls=calls, total_cols=total_cols, idx_flat=idx_flat)


# ----------------------------------------------------------------------------
# device program
# ----------------------------------------------------------------------------

def _build_program(calls, total_cols):
    nc = bacc.Bacc("TRN2", target_bir_lowering=False, debug=False,
                   num_devices=NCOR, num_swdge_queues=NQUEUES)
    f32, bf16, i16 = mybir.dt.float32, mybir.dt.bfloat16, mybir.dt.int16

    t_xtab = nc.dram_tensor("xtab", [2 * HALF, F_IN], bf16,
                            kind="ExternalInput")
    t_xT = nc.dram_tensor("xT", [128, PADN], bf16, kind="ExternalInput")
    t_idx = nc.dram_tensor("idx", [128 * total_cols * 8], i16,
                           kind="ExternalInput")
    wnames = ["Wl1", "Wr1", "Wl2", "Wr2", "Wla", "Wra", "Wlm", "Wrm"]
    wcols = {"Wl1": H, "Wr1": H}
    t_w = {w: nc.dram_tensor(w, [128, wcols.get(w, 2 * H)], bf16,
                             kind="ExternalInput") for w in wnames}
    t_b = {b: nc.dram_tensor(b, [128, 2], f32, kind="ExternalInput")
           for b in ["bl1", "bl2", "bla", "blm"]}
    t_wh = {w: nc.dram_tensor(w, [128, 2], bf16, kind="ExternalInput")
            for w in ["Wa", "Wm"]}
    t_bh = {b: nc.dram_tensor(b, [1, 1], f32, kind="ExternalInput")
            for b in ["ba", "bm"]}
    t_out = nc.dram_tensor("out", [2, NLOC], f32, kind="ExternalOutput")

    NCH = (TILES + CHUNK - 1) // CHUNK
    cw_of = lambda c: min(CHUNK, TILES - c * CHUNK) * 128
    calls_of_chunk = {}
    for ci, (phase, col0, ncols, c, spans) in enumerate(calls):
        calls_of_chunk.setdefault(c, []).append(ci)

    qctr = [0]

    with tile.TileContext(nc) as tc:
        with tc.tile_pool(name="const", bufs=1) as cpool, \
             tc.tile_pool(name="hT", bufs=1) as hpool, \
             tc.tile_pool(name="gat", bufs=8) as gp, \
             tc.tile_pool(name="work", bufs=3) as wk, \
             tc.tile_pool(name="psT", bufs=3, space="PSUM") as psT, \
             tc.tile_pool(name="psY", bufs=3, space="PSUM") as psY, \
             tc.tile_pool(name="dram", bufs=1, space="DRAM") as dram:

            # shared gather index stream (same for all three layers).
            # Loaded first: it gates the first gather; the first chunk's
            # columns come in a small leading DMA.
            idx_sb = hpool.tile([128, total_cols * 8], i16, name="idx_sb")
            c0_cols = max(c[1] + c[2] for c in calls if c[3] == 0) * 8
            idx2d = t_idx[:].rearrange("(p s) -> p s", p=128)
            nc.sync.dma_start(idx_sb[:, :c0_cols], idx2d[:, :c0_cols])
            nc.sync.dma_start(idx_sb[:, c0_cols:], idx2d[:, c0_cols:])

            ident = cpool.tile([128, 128], bf16, name="ident")
            make_identity(nc, ident[:])

            w_sb = {}
            for w in wnames:
                cols = wcols.get(w, 2 * H)
                ws = cpool.tile([128, cols], bf16, name=f"sb_{w}")
                nc.sync.dma_start(ws[:], t_w[w][:])
                w_sb[w] = ws
            b_sb = {}
            for b in t_b:
                bs = cpool.tile([128, 2], f32, name=f"sb_{b}")
                nc.sync.dma_start(bs[:], t_b[b][:])
                b_sb[b] = bs
            wh_sb = {}
            for w in t_wh:
                ws = cpool.tile([128, 2], bf16, name=f"sb_{w}")
                nc.sync.dma_start(ws[:], t_wh[w][:])
                wh_sb[w] = ws
            bh_sb = {}
            for b in t_bh:
                bs = cpool.tile([1, 1], f32, name=f"sb_{b}")
                nc.sync.dma_start(bs[:], t_bh[b][:])
                bh_sb[b] = bs

            xT_sb = hpool.tile([128, PADN], bf16, name="xT_sb")
            nc.sync.dma_start(xT_sb[:], t_xT[:])
            h1T = hpool.tile([128, 2 * PADN], bf16, name="h1T")
            h2T = hpool.tile([128, 2 * PADN], bf16, name="h2T")

            h1tab = dram.tile([2 * HALF, H], bf16, name="h1tab",
                              addr_space="Shared")
            h2tab = dram.tile([2 * HALF, H], bf16, name="h2tab",
                              addr_space="Shared")
            blk1 = dram.tile([BLOCK, H], bf16, name="blk1")
            blk2 = dram.tile([BLOCK, H], bf16, name="blk2")

            # each core's block ends with a -inf pad row
            padrow = cpool.tile([1, H], bf16, name="padrow")
            nc.vector.memset(padrow[:], NEG)
            nc.sync.dma_start(blk1[NLOC:NLOC + 1, :], padrow[:])
            nc.sync.dma_start(blk2[NLOC:NLOC + 1, :], padrow[:])

            def aggregate_chunk(c, table, F, tag):
                """Gather + max-reduce chunk c's neighbors from `table`.
                Node-major gathers (multi-queue safe), per-span strided
                DVE max-reduce, then PE-transpose the reduced [128, F]
                aggregate. Returns bf16 f-major agg tile [128, fh*512]."""
                fh = F // 128
                agg = wk.tile([128, fh * 512], bf16, name=f"agg_{tag}",
                              tag="agg")
                aggn = {}        # tile -> node-major agg tile [128, F]
                for ci in calls_of_chunk[c]:
                    phase, col0, ncols, _, spans = calls[ci]
                    nidx = ncols * 128
                    g = gp.tile([128, KCOL * F], bf16,
                                name=f"g_{tag}_{ci}", tag="g")
                    view = table[0:HALF, :] if phase == 0 \
                        else table[HALF:2 * HALF, :]
                    nc.gpsimd.dma_gather(
                        out_ap=g[:, :ncols * F].rearrange(
                            "p (k f) -> p k f", f=F),
                        in_ap=view,
                        idxs_ap=idx_sb[:, col0 * 8:(col0 + ncols) * 8],
                        num_idxs=nidx, num_idxs_reg=nidx, elem_size=F,
                        single_packet=False,
                        queue_num=qctr[0] % NQUEUES)
                    qctr[0] += 1
                    for (t, k0, kn, off) in spans:
                        # one contiguous pair-fold (halves the columns),
                        # then a single strided max-reduce over the rest
                        k = kn
                        if k > 2 and k % 2 == 1:
                            nc.vector.tensor_tensor(
                                out=g[:, off * F:(off + 1) * F],
                                in0=g[:, off * F:(off + 1) * F],
                                in1=g[:, (off + k - 1) * F:(off + k) * F],
                                op=mybir.AluOpType.max)
                            k -= 1
                        if k > 2:
                            half = k // 2
                            nc.vector.tensor_tensor(
                                out=g[:, off * F:(off + half) * F],
                                in0=g[:, off * F:(off + half) * F],
                                in1=g[:, (off + half) * F:
                                       (off + 2 * half) * F],
                                op=mybir.AluOpType.max)
                            k = half
                        src = g[:, off * F:(off + k) * F]
                        view3 = src.rearrange("p (k f) -> p f k", f=F)
                        if t not in aggn:
                            an = wk.tile([128, F], bf16,
                                         name=f"an_{tag}_{t}", tag="aggn",
                                         bufs=10)
                            aggn[t] = an
                            nc.vector.tensor_reduce(
                                out=an[:], in_=view3,
                                axis=mybir.AxisListType.X,
                                op=mybir.AluOpType.max)
                        else:
                            an = aggn[t]
                            tmp = wk.tile([128, F], bf16,
                                          name=f"tmp_{tag}", tag="tmp")
                            nc.vector.tensor_reduce(
                                out=tmp[:], in_=view3,
                                axis=mybir.AxisListType.X,
                                op=mybir.AluOpType.max)
                            nc.vector.tensor_tensor(
                                out=an[:], in0=an[:], in1=tmp[:],
                                op=mybir.AluOpType.max)
                for t, an in aggn.items():
                    toff = (t - c * CHUNK) * 128
                    for p in range(fh):
                        tp = psT.tile([128, 128], bf16, name=f"tpa_{tag}",
                                      tag="tp")
                        nc.tensor.transpose(
                            tp[:], an[:, p * 128:(p + 1) * 128], ident[:])
                        nc.scalar.activation(
                            agg[:, p * 512 + toff:p * 512 + toff + 128],
                            tp[:],
                            mybir.ActivationFunctionType.Identity)
                return agg

            def write_table(yT, c, blkout, tag):
                """yT bf16 [128, 2*PADN] planes -> node-major rows of blkout
                for chunk c (PE transpose + ACT copy)."""
                cw = cw_of(c)
                for i in range(cw // 128):
                    t = c * CHUNK + i
                    ynode = wk.tile([128, H], bf16, name=f"yn_{tag}",
                                    tag="ynode")
                    for hh in range(2):
                        tp = psT.tile([128, 128], bf16, name=f"tpo_{tag}",
                                      tag="tp")
                        nc.tensor.transpose(
                            tp[:],
                            yT[:, hh * PADN + t * 128:hh * PADN + (t + 1) * 128],
                            ident[:])
                        nc.scalar.activation(
                            ynode[:, hh * 128:(hh + 1) * 128], tp[:],
                            mybir.ActivationFunctionType.Identity)
                    rows = min(128, NLOC - t * 128)
                    nc.sync.dma_start(blkout[t * 128:t * 128 + rows, :],
                                      ynode[:rows, :])

            def layer(table, selfT, F, Wl, Wr, bl, outT, blkout, tag):
                fh = F // 128
                for c in range(NCH):
                    cw = cw_of(c)
                    c0 = c * CHUNK * 128
                    agg = aggregate_chunk(c, table, F, f"{tag}_{c}")
                    for hh in range(2):
                        psy = psY.tile([128, 512], f32, name=f"psy_{tag}",
                                       tag="psy")
                        nmm = 2 * fh
                        i = 0
                        for p in range(fh):
                            nc.tensor.matmul(
                                psy[:, :cw],
                                w_sb[Wl][:, p * H + hh * 128:
                                         p * H + (hh + 1) * 128],
                                agg[:, p * 512:p * 512 + cw],
                                start=(i == 0), stop=(i == nmm - 1))
                            i += 1
                            nc.tensor.matmul(
                                psy[:, :cw],
                                w_sb[Wr][:, p * H + hh * 128:
                                         p * H + (hh + 1) * 128],
                                selfT[:, p * PADN + c0:p * PADN + c0 + cw],
                                start=(i == 0), stop=(i == nmm - 1))
                            i += 1
                        nc.scalar.activation(
                            outT[:, hh * PADN + c0:hh * PADN + c0 + cw],
                            psy[:, :cw],
                            mybir.ActivationFunctionType.Relu,
                            bias=b_sb[bl][:, hh:hh + 1])
                    write_table(outT, c, blkout, tag)

            layer(t_xtab, xT_sb, F_IN, "Wl1", "Wr1", "bl1", h1T, blk1, "l1")
            nc.gpsimd.collective_compute(
                "AllGather", mybir.AluOpType.bypass,
                replica_groups=[list(range(NCOR))],
                ins=[blk1.opt()], outs=[h1tab.opt()])
            layer(h1tab, h1T, H, "Wl2", "Wr2", "bl2", h2T, blk2, "l2")
            nc.gpsimd.collective_compute(
                "AllGather", mybir.AluOpType.bypass,
                replica_groups=[list(range(NCOR))],
                ins=[blk2.opt()], outs=[h2tab.opt()])

            # layer 3: two branches + heads (shared aggregation)
            for c in range(NCH):
                cw = cw_of(c)
                c0 = c * CHUNK * 128
                agg = aggregate_chunk(c, h2tab, H, f"l3_{c}")
                out_sbs = [wk.tile([1, 512], f32, name=f"out_sb{bi}",
                                   tag=f"out_sb{bi}") for bi in range(2)]
                for bi, (Wl, Wr, bl, Wh, bh) in enumerate(
                        [("Wla", "Wra", "bla", "Wa", "ba"),
                         ("Wlm", "Wrm", "blm", "Wm", "bm")]):
                    brT = wk.tile([128, 2 * 512], bf16, name=f"brT{bi}",
                                  tag="brT")
                    for hh in range(2):
                        psy = psY.tile([128, 512], f32, name=f"psy3_{bi}",
                                       tag="psy")
                        for p in range(2):
                            nc.tensor.matmul(
                                psy[:, :cw],
                                w_sb[Wl][:, p * H + hh * 128:
                                         p * H + (hh + 1) * 128],
                                agg[:, p * 512:p * 512 + cw],
                                start=(p == 0), stop=False)
                            nc.tensor.matmul(
                                psy[:, :cw],
                                w_sb[Wr][:, p * H + hh * 128:
                                         p * H + (hh + 1) * 128],
                                h2T[:, p * PADN + c0:p * PADN + c0 + cw],
                                start=False, stop=(p == 1))
                        nc.scalar.activation(
                            brT[:, hh * 512:hh * 512 + cw], psy[:, :cw],
                            mybir.ActivationFunctionType.Relu,
                            bias=b_sb[bl][:, hh:hh + 1])
                    psh = psY.tile([1, 512], f32, name=f"psh{bi}", tag="psh",
                                   bufs=2)
                    for hh in range(2):
                        nc.tensor.matmul(psh[:, :cw],
                                         wh_sb[Wh][:, hh:hh + 1],
                                         brT[:, hh * 512:hh * 512 + cw],
                                         start=(hh == 0), stop=(hh == 1))
                    nc.scalar.activation(out_sbs[bi][:, :cw],
                                         psh[:, :cw],
                                         mybir.ActivationFunctionType.Identity,
                                         bias=bh_sb[bh][:])
                live = min(cw, NLOC - c0)
                for bi in range(2):
                    nc.sync.dma_start(
                        t_out[bi:bi + 1, c0:c0 + live],
                        out_sbs[bi][:, :live])

    nc.compile()
    return nc


# ----------------------------------------------------------------------------
# entry point
# ----------------------------------------------------------------------------

def kernel(x, edge_index, Wl1, bl1, Wr1, Wl2, bl2, Wr2,
           Wla, bla, Wra, Wa, ba, Wlm, blm, Wrm, Wm, bm):
    x = np.asarray(x, np.float32)
    pp = _preprocess(edge_index)
    old_of_new = pp["old_of_new"]

    # x gather table in block layout: per core 6250 rows + one -inf pad row
    xp = x[old_of_new]
    xtab = np.empty((2 * HALF, F_IN), np.float32)
    for m in range(NCOR):
        base = m * BLOCK if m < 4 else HALF + (m - 4) * BLOCK
        xtab[base:base + NLOC] = xp[m * NLOC:(m + 1) * NLOC]
        xtab[base + NLOC] = NEG
    xtab = xtab.astype(ml_dtypes.bfloat16)

    nc = _build_program(pp["calls"], pp["total_cols"])

    def wchunk(W, fi):
        """[fi, H] f32 -> bf16 [128, (fi//128)*H] chunk-major stationary."""
        W = np.asarray(W, np.float32)
        fh = fi // 128
        out = np.empty((128, fh * H), np.float32)
        for c in range(fh):
            out[:, c * H:(c + 1) * H] = W[c * 128:(c + 1) * 128, :]
        return out.astype(ml_dtypes.bfloat16)

    def bchunk(b):
        return np.ascontiguousarray(
            np.asarray(b, np.float32).reshape(2, 128).T)

    w_ins = {
        "Wl1": wchunk(Wl1, F_IN), "Wr1": wchunk(Wr1, F_IN),
        "Wl2": wchunk(Wl2, H), "Wr2": wchunk(Wr2, H),
        "Wla": wchunk(Wla, H), "Wra": wchunk(Wra, H),
        "Wlm": wchunk(Wlm, H), "Wrm": wchunk(Wrm, H),
        "Wa": np.asarray(Wa, np.float32).reshape(2, 128).T.astype(
            ml_dtypes.bfloat16).copy(),
        "Wm": np.asarray(Wm, np.float32).reshape(2, 128).T.astype(
            ml_dtypes.bfloat16).copy(),
        "bl1": bchunk(bl1), "bl2": bchunk(bl2),
        "bla": bchunk(bla), "blm": bchunk(blm),
        "ba": np.asarray(ba, np.float32).reshape(1, 1),
        "bm": np.asarray(bm, np.float32).reshape(1, 1),
    }

    in_maps = []
    for m in range(NCOR):
        blk = xp[m * NLOC:(m + 1) * NLOC]
        xT = np.zeros((128, PADN), np.float32)
        xT[:, :NLOC] = blk.T
        in_maps.append({
            "xtab": xtab, "xT": xT.astype(ml_dtypes.bfloat16),
            "idx": pp["idx_flat"][m], **w_ins,
        })

    res = run_bass_kernel_spmd(nc, in_maps, core_ids=list(range(NCOR)))

    rt = np.empty(N, np.float32)
    mv = np.empty(N, np.float32)
    for m in range(NCOR):
        out = res.results[m]["out"]
        rt[m * NLOC:(m + 1) * NLOC] = out[0]
        mv[m * NLOC:(m + 1) * NLOC] = out[1]
    rt_o = np.empty(N, np.float32)
    mv_o = np.empty(N, np.float32)
    rt_o[old_of_new] = rt
    mv_o[old_of_new] = mv

    _LAST.update(nc=nc, in_maps=in_maps, pp=pp)
    return (rt_o, mv_o)
